# revision 1
# baseline (speedup 1.0000x reference)
"""BERT self-attention on 8 Trainium2 NeuronCores (Bass/Tile), v2.

Sharding: tensor-parallel over heads. Core c owns heads {2c, 2c+1} =
output columns [128c, 128c+128). Every core reads the full (host
pre-transposed, fp16) hidden_states; the host concatenates the 8
per-core [B*S, 128] outputs and adds bv (softmax weights sum to 1, so
the V bias is a constant output shift).

Per-core pipeline (B=4, S=2048, D=1024, head_dim=64), all-fp16 matmul
operands (fp32 PSUM accumulation; measured end-to-end rel err ~1e-3):

  QKV(b):  Q^T/K^T [dpc=128, t] via stationary-W matmuls (moving X^T,
    N=512); V directly in [t, d] layout via stationary-X^T matmuls
    (moving Wv, N=128) -- no transposes anywhere. PSUM->SBUF copies on
    DVE add the q/k biases and fold exp(attention_mask) into V rows
    plus a per-head exp(mask) column for the softmax denominators.

  attn(b, h, qc):  scores S^T[k, q] as K=64 single-shot matmuls
    (N=512); exp on ACT over [128, 2, 512] PSUM tiles -> es fp16, with
    3-5 of 16 k-blocks computed instead via a phase-averaged
    Schraudolph fast-exp on DVE (one int16 magic-constant convert, an
    exact integer -512 phase shift, and an fp16 sum -- folded into PV
    as a second accumulating matmul for the last batch, whose units
    have no next-batch projection work to fill PE slack) to keep ACT
    under the PE roofline; PV with the big es operand STATIONARY
    ([128 k, 128 q]) and v_aug [128 k, 65] moving (ones column ->
    denominators); all 4 q-tile chains of a pv tile form a single
    psum accumulation group (one 2KB lazy-zero region allows only one
    open group); DVE reciprocal + broadcast-multiply into a per-batch
    staging tile; one [128, 4, 128] DMA per 512 rows (dram side
    rearranged so element orders match).

The PE instruction stream is software-pipelined at ~0.5us quantum
granularity (engines execute in order, so any instruction waiting on
a PSUM ring slot head-blocks everything behind it): each k-block slot
pops a next-batch QKV quantum and a PV stripe step (PV_LAG slots
behind exp, consuming each es tile as soon as it exists), then emits
2 score matmuls + the exp; normalize quanta are deferred behind the
PSUM-releasing converts in DVE's queue. Batch 0 runs a minimal k/q
prefix with deadline-ordered fillers; DMA issue order is matched to
the consumption chain.
"""

from collections import deque

import numpy as np

import concourse.bass as bass
import concourse.tile as tile
from concourse import bacc, mybir
from concourse.bass_utils import run_bass_kernel_spmd

B, S, D, H = 4, 2048, 1024, 16
DH = 64
N_CORES = 8
DPC = D // N_CORES  # 128 output dims (2 heads) per core
BS = B * S  # 8192

F32 = mybir.dt.float32
F16 = mybir.dt.float16
I16 = mybir.dt.int16

# Phase-averaged Schraudolph fast-exp (tuned vs numpy on the reference
# distribution): es = f16bits(round(p*A + B1)) + f16bits(round(p*A + B2))
# where p is the raw (unscaled) score. A folds the 1/sqrt(dh) scale and
# log2(e) into the fp16 exponent step; B2 = B1 - 512 phase-shifts the
# interpolation sawtooth half a period; both are shifted log2(1.7766) so
# the pair sums to an unbiased exp estimate (+-1.1% weight error).
import os

if os.environ.get("NOSCH"):
    SCHRAUD_KB = ()  # which of 16 k-blocks per (h, qc) use fast-exp
    SCHRAUD_KB_STARVED = ()
    SCHRAUD_KB_TAIL = ()
else:
    SCHRAUD_KB = (4, 9, 14)
    # last batch has no next-batch QKV filler for the PE, so its units are
    # exp-drain-paced: offload more k-blocks there; their phase-pair sum is
    # folded into PV as two accumulating matmuls (PE slack absorbs it)
    SCHRAUD_KB_STARVED = (1, 4, 7, 10, 13)
    SCHRAUD_KB_TAIL = (1, 3, 5, 8, 10)  # keep the final k-blocks on ACT
SCH_A = 0.125 * 1.4426950408889634 * 1024.0
SCH_B1 = 14.170916 * 1024.0
SCH_B2 = 13.670916 * 1024.0

_CACHE: dict = {}


def _build(use_mask: bool):
    nc = bacc.Bacc(
        "TRN2", target_bir_lowering=False, debug=False, enable_asserts=False
    )

    xtd = nc.dram_tensor("xt", [D, BS], F16, kind="ExternalInput").ap()
    wq = nc.dram_tensor("wq", [D, DPC], F16, kind="ExternalInput").ap()
    wk = nc.dram_tensor("wk", [D, DPC], F16, kind="ExternalInput").ap()
    wv = nc.dram_tensor("wv", [D, DPC], F16, kind="ExternalInput").ap()
    bq = nc.dram_tensor("bq", [DPC], F32, kind="ExternalInput").ap()
    bk = nc.dram_tensor("bk", [DPC], F32, kind="ExternalInput").ap()
    msk = nc.dram_tensor("msk", [B, S], F32, kind="ExternalInput").ap()
    out = nc.dram_tensor("out", [BS, DPC], F32, kind="ExternalOutput").ap()

    Exp = mybir.ActivationFunctionType.Exp
    Add = mybir.AluOpType.add
    Mult = mybir.AluOpType.mult

    with tile.TileContext(nc) as tc:
        with (
            tc.tile_pool(name="consts", bufs=1) as consts,
            tc.tile_pool(name="p_xt", bufs=6) as p_xt,
            tc.tile_pool(name="p_qk", bufs=2) as p_qk,
            tc.tile_pool(name="p_v", bufs=2) as p_v,
            tc.tile_pool(name="p_es", bufs=33) as p_es,
            tc.tile_pool(name="p_fin", bufs=2) as p_fin,
            tc.tile_pool(name="p_sm", bufs=8) as p_sm,
            tc.tile_pool(name="ps_qk", bufs=2, space="PSUM") as ps_qk,
            tc.tile_pool(name="ps_sp", bufs=2, space="PSUM") as ps_sp,
            tc.tile_pool(name="ps_pv", bufs=2, space="PSUM") as ps_pv,
        ):
            # ---- DMA issue order follows the consumption order (the DMA
            # engines transfer strictly in issue order): wk + batch 0's four
            # X^T chunks first (k-projections), only then wq/wv/bias/mask ----
            wk_sb = consts.tile([128, 8, DPC], F16, tag="wk_sb")
            wk_src = wk.rearrange("(cc p) d -> p cc d", p=128)

            xts: dict = {}

            def load_xt(b, tch, half=None):
                """X^T chunk in two half-transfers so the first 4-matmul
                projection group can start after half a chunk."""
                t0 = b * S + tch * 512
                if half is None or half == 0:
                    xt = p_xt.tile(
                        [128, 8, 512], F16, tag="xt", name=f"xt{b}_{tch}"
                    )
                    xts[(b, tch)] = xt
                xt = xts[(b, tch)]
                src = xtd[:, t0 : t0 + 512].rearrange("(cc p) t -> p cc t", p=128)
                if half != 1:
                    nc.sync.dma_start(out=xt[:, 0:4, :], in_=src[:, 0:4, :])
                if half != 0:
                    nc.sync.dma_start(out=xt[:, 4:8, :], in_=src[:, 4:8, :])

            # interleave the weight/X transfers to match the k0a -> k0b ->
            # q0 -> k1 consumption chain of batch 0's projection prefix
            bq_sb = consts.tile([128, 1], F32, tag="bq_sb")
            bk_sb = consts.tile([128, 1], F32, tag="bk_sb")
            nc.sync.dma_start(out=wk_sb[:, 0:4, :], in_=wk_src[:, 0:4, :])
            load_xt(0, 0, half=0)
            nc.sync.dma_start(out=bk_sb, in_=bk.rearrange("(p o) -> p o", o=1))
            nc.sync.dma_start(out=bq_sb, in_=bq.rearrange("(p o) -> p o", o=1))
            nc.sync.dma_start(out=wk_sb[:, 4:8, :], in_=wk_src[:, 4:8, :])
            load_xt(0, 0, half=1)

            wq_sb = consts.tile([128, 8, DPC], F16, tag="wq_sb")
            wv_sb = consts.tile([128, 8, DPC], F16, tag="wv_sb")
            nc.sync.dma_start(out=wq_sb, in_=wq.rearrange("(cc p) d -> p cc d", p=128))
            load_xt(0, 1)
            nc.sync.dma_start(out=wv_sb, in_=wv.rearrange("(cc p) d -> p cc d", p=128))

            load_xt(0, 2)
            load_xt(0, 3)

            if use_mask:
                m_sb = consts.tile([128, B, 16], F32, tag="m_sb")
                nc.sync.dma_start(
                    out=m_sb, in_=msk.rearrange("b (kb p) -> p b kb", p=128)
                )
                emask = consts.tile([128, B, 16], F32, tag="emask")
                nc.scalar.activation(emask, m_sb, Exp)

            # per-b state handles
            qT: dict = {}
            kT: dict = {}
            v_sb: dict = {}
            fin: dict = {}
            accs: dict = {}

            qkv_q: deque = deque()

            def push_qkv(b):
                """Queue batch b's projection work as ~0.5us PE quanta.
                K projections first so the next attention unit's scores
                can start as early as possible."""

                def alloc(b=b):
                    qT[b] = p_qk.tile([128, 4, 512], F16, tag="qT", name=f"qT{b}")
                    kT[b] = p_qk.tile([128, 4, 512], F16, tag="kT", name=f"kT{b}")
                    v_sb[b] = p_v.tile(
                        [128, 16, 2, DH + 1], F16, tag="v_sb", name=f"v{b}"
                    )
                    # softmax-denominator / mask column for every k-tile
                    if use_mask:
                        nc.vector.tensor_copy(
                            v_sb[b][:, :, :, DH : DH + 1],
                            emask[:, b, :]
                            .unsqueeze(2)
                            .unsqueeze(3)
                            .broadcast_to([128, 16, 2, 1]),
                        )
                    else:
                        nc.vector.memset(v_sb[b][:, :, :, DH : DH + 1], 1.0)

                qkv_q.append((b, alloc))

                def proj_half(b, tch, w_sb, which, half):
                    """4 accumulating matmuls (half the D contraction)."""
                    key = (b, tch, which)
                    if half == 0:
                        accs[key] = ps_qk.tile(
                            [128, 512], F32, tag="qk", name=f"acc{which}{b}{tch}"
                        )
                    acc = accs[key]
                    xt = xts[(b, tch)]
                    for cc in range(4 * half, 4 * half + 4):
                        nc.tensor.matmul(
                            acc,
                            w_sb[:, cc, :],
                            xt[:, cc, :],
                            start=(cc == 0),
                            stop=(cc == 7),
                        )
                    if half == 1:
                        dst, b_sb = (
                            (kT[b], bk_sb) if which == "k" else (qT[b], bq_sb)
                        )
                        nc.vector.tensor_scalar_add(dst[:, tch, :], acc, b_sb)

                def v_tile(b, tch, tt):
                    """One [128 t, 128 d] V tile: 8 matmuls + masked copy.
                    All 4 t-tiles of the chunk share the tile's single 2KB
                    zero region, so they form ONE accumulation group."""
                    key = (b, tch, "v")
                    if tt == 0:
                        accs[key] = ps_qk.tile(
                            [128, 512], F32, tag="qk", name=f"accv{b}{tch}"
                        )
                    vps = accs[key]
                    xt = xts[(b, tch)]
                    o = vps[:, tt * 128 : (tt + 1) * 128]
                    for cc in range(8):
                        nc.tensor.matmul(
                            o,
                            xt[:, cc, tt * 128 : (tt + 1) * 128],
                            wv_sb[:, cc, :],
                            start=(tt == 0 and cc == 0),
                            stop=(tt == 3 and cc == 7),
                        )
                    if tt < 3:
                        return  # copies only after the psum group closes
                    t0 = tch * 4
                    if use_mask:
                        for i in range(4):
                            nc.vector.tensor_scalar_mul(
                                v_sb[b][:, t0 + i, :, 0:DH],
                                vps[:, i * 128 : (i + 1) * 128].rearrange(
                                    "p (h d) -> p h d", h=2
                                ),
                                emask[:, b, t0 + i : t0 + i + 1],
                            )
                    else:
                        for i in range(4):
                            nc.vector.tensor_copy(
                                v_sb[b][:, t0 + i, :, 0:DH],
                                vps[:, i * 128 : (i + 1) * 128].rearrange(
                                    "p (h d) -> p h d", h=2
                                ),
                            )

                if b == 0:
                    # minimal prefix for unit (0,0,0): k/q of chunks 0-1
                    # (drained upfront); everything else becomes unit-0 slot
                    # fillers ordered by first use (v0 for the PV stripe,
                    # k2/k3 before their score slots, then v1/q2/v2; q3/v3
                    # drain at unit end) so the first exp starts ~14us earlier
                    def ph(t, w, which, half):
                        return (0, lambda: proj_half(0, t, w, which, half))

                    def vt(t, tt):
                        return (0, lambda: v_tile(0, t, tt))

                    qkv_q.extend([ph(0, wk_sb, "k", 0), ph(0, wk_sb, "k", 1)])
                    qkv_q.extend([ph(0, wq_sb, "q", 0), ph(0, wq_sb, "q", 1)])
                    qkv_q.extend([ph(1, wk_sb, "k", 0), ph(1, wk_sb, "k", 1)])
                    qkv_q.extend([ph(1, wq_sb, "q", 0), ph(1, wq_sb, "q", 1)])
                    # ---- fillers from here (popped 1/slot inside unit 0,
                    # ordered by deadline: scores(kb) needs kT[kb//4] by slot
                    # kb; pv_step(kb) at slot kb+8 needs the chunk's LAST v
                    # quantum (which carries the psum->sbuf copies) first ----
                    qkv_q.extend([vt(0, tt) for tt in range(4)])  # slots 0-3
                    qkv_q.extend([ph(2, wk_sb, "k", 0), ph(2, wk_sb, "k", 1)])
                    qkv_q.extend([vt(1, tt) for tt in range(4)])  # slots 6-9
                    qkv_q.extend([ph(3, wk_sb, "k", 0), ph(3, wk_sb, "k", 1)])
                    qkv_q.extend([vt(2, tt) for tt in range(4)])  # slots 12-15
                    # drained at unit-0 end, before its post-loop pv steps:
                    qkv_q.extend([ph(2, wq_sb, "q", 0), ph(2, wq_sb, "q", 1)])
                    qkv_q.extend([ph(3, wq_sb, "q", 0), ph(3, wq_sb, "q", 1)])
                    qkv_q.extend([vt(3, tt) for tt in range(4)])
                    return
                # chunk-major so the prefetch tracks the DMA arrival order
                for tch in range(4):
                    qkv_q.append((b, lambda b=b, t=tch: load_xt(b, t)))
                    qkv_q.append((b, lambda b=b, t=tch: proj_half(b, t, wk_sb, "k", 0)))
                    qkv_q.append((b, lambda b=b, t=tch: proj_half(b, t, wk_sb, "k", 1)))
                    qkv_q.append((b, lambda b=b, t=tch: proj_half(b, t, wq_sb, "q", 0)))
                    qkv_q.append((b, lambda b=b, t=tch: proj_half(b, t, wq_sb, "q", 1)))
                    for tt in range(4):
                        qkv_q.append((b, lambda b=b, t=tch, tt=tt: v_tile(b, t, tt)))

            PV_LAG = 8  # slots between exp(kb) and its striped PV step
            dve_q: deque = deque()  # deferred normalize/ship quanta

            def emit_unit(b, h, qc, schraud, lag=PV_LAG):
                """One (batch, head, 1024-q-chunk) attention unit as a
                single slot pipeline: per k-block 2 score matmuls + exp,
                the PV accumulation STRIPED kb-wise PV_LAG slots behind
                (each es tile is consumed as soon as it exists), and one
                next-batch QKV quantum per slot."""
                hp = h * DH
                if h == 0 and qc == 0:
                    fin[b] = p_fin.tile([128, 16, DPC], F32, tag="fin", name=f"f{b}")
                pvt = [
                    ps_pv.tile([128, 4, DH + 1], F32, tag="pv", name=f"pv{j}")
                    for j in range(2)
                ]
                tiles = []

                def pv_step(kb):
                    # all 4 q-tile chains of one pv tile form a SINGLE psum
                    # accumulation group (one 2KB zero region): start only on
                    # the region's very first matmul, stop on its last —
                    # later chains' first writes consume the pending-zero
                    parts = tiles[kb]
                    for qt in range(8):
                        for pi, part in enumerate(parts):
                            nc.tensor.matmul(
                                pvt[qt // 4][:, qt % 4, :],
                                part[:, qt // 4, (qt % 4) * 128 : (qt % 4 + 1) * 128],
                                v_sb[b][:, kb, h, :],
                                start=(kb == 0 and pi == 0 and qt % 4 == 0),
                                stop=(
                                    kb == 15
                                    and pi == len(parts) - 1
                                    and qt % 4 == 3
                                ),
                            )

                pv_sum = b == B - 1  # starved batch: PE slack absorbs the sum
                for kb in range(16):
                    # QKV + PV quanta lead the slot so a scores matmul that
                    # must wait for its PSUM ring slot (ACT/DVE drain) sits
                    # behind ready work instead of head-blocking it (QKV pop
                    # first: unit-0 fillers write the very tiles pv_step reads)
                    if qkv_q and (kb % 2 == 0 or len(qkv_q) > 33):
                        qkv_q.popleft()[1]()
                    if kb >= lag:
                        pv_step(kb - lag)
                    sp = ps_sp.tile([128, 2, 512], F32, tag="sp")
                    for j in range(2):
                        nc.tensor.matmul(
                            sp[:, j, :],
                            kT[b][
                                hp : hp + DH,
                                kb // 4,
                                (kb % 4) * 128 : (kb % 4 + 1) * 128,
                            ],
                            qT[b][hp : hp + DH, 2 * qc + j, :],
                            start=True,
                            stop=True,
                        )
                    es = p_es.tile(
                        [128, 2, 512], F16, tag="es", name=f"es{kb}", bufs=14
                    )
                    if kb in schraud:
                        # one PSUM read: u1 = round(p*A + B1) as int16 bits;
                        # the half-period phase shift is an exact integer
                        # subtract (B1-B2 = 512 = half the fp16 exponent
                        # step). The phase-pair sum runs on DVE, or (starved
                        # batches) folds into PV as a 2nd accumulating matmul.
                        u2 = p_es.tile(
                            [128, 2, 512], I16, tag="sch", name="u2", bufs=6
                        )
                        nc.vector.tensor_scalar(
                            es.bitcast(I16), sp, SCH_A, SCH_B1, op0=Mult, op1=Add
                        )
                        nc.vector.tensor_scalar(
                            u2, es.bitcast(I16), 512, None,
                            op0=mybir.AluOpType.subtract,
                        )
                        if pv_sum:
                            tiles.append((es, u2.bitcast(F16)))
                        else:
                            nc.vector.tensor_tensor(es, es, u2.bitcast(F16), Add)
                            tiles.append((es,))
                    else:
                        nc.scalar.activation(es, sp, Exp, scale=0.125)
                        tiles.append((es,))
                    # deferred norms run AFTER this slot's exp ops so they
                    # never delay the PSUM-releasing converts in DVE's queue
                    if dve_q:
                        dve_q.popleft()()
                # all same-batch projection writes must be emitted before the
                # remaining PV steps read them (batch 0's deferred v tiles)
                while qkv_q and qkv_q[0][0] == b:
                    qkv_q.popleft()[1]()
                for kb in range(16 - lag, 16):
                    pv_step(kb)

                def norm_half(half, b=b, h=h, qc=qc, pvt=pvt):
                    pv = pvt[half]
                    rc = p_sm.tile([128, 4, 1], F32, tag="rc")
                    nc.vector.reciprocal(rc, pv[:, :, DH : DH + 1])
                    qi0 = 8 * qc + 4 * half
                    nc.vector.tensor_tensor(
                        fin[b][:, qi0 : qi0 + 4, h * DH : (h + 1) * DH],
                        pv[:, :, 0:DH],
                        rc.broadcast_to([128, 4, DH]),
                        Mult,
                    )
                    if h == 1:  # both heads done: ship these 4 q-tiles
                        q0 = b * S + qc * 1024 + half * 512
                        nc.sync.dma_start(
                            out=out[q0 : q0 + 512, :].rearrange(
                                "(t p) d -> p t d", p=128
                            ),
                            in_=fin[b][:, qi0 : qi0 + 4, :],
                        )

                dve_q.append(lambda: norm_half(0))
                dve_q.append(lambda: norm_half(1))

            # ---- software-pipelined emission ----
            push_qkv(0)
            for _ in range(9):  # alloc + k(chunks 0-1) + q(chunks 0-1)
                qkv_q.popleft()[1]()
            units = [(b, h, qc) for b in range(B) for h in range(2) for qc in range(2)]
            for i, (b, h, qc) in enumerate(units):
                if h == 0 and qc == 0:
                    # batch b's projections must be fully emitted before any
                    # instruction reading qT/kT/v_sb[b] (Tile deps track
                    # writes-before-reads in program order); b=0 defers its
                    # v/q23 quanta into unit 0's slots instead
                    if b > 0:
                        while qkv_q and qkv_q[0][0] == b:
                            qkv_q.popleft()[1]()
                    if b + 1 < B:
                        push_qkv(b + 1)
                if i == len(units) - 1:
                    sch = SCHRAUD_KB_TAIL
                elif b == B - 1:
                    sch = SCHRAUD_KB_STARVED
                else:
                    sch = SCHRAUD_KB
                emit_unit(b, h, qc, sch, lag=8 if i == 0 else PV_LAG)
            while dve_q:
                dve_q.popleft()()
            while qkv_q:
                qkv_q.popleft()[1]()

    nc.compile()
    return nc


def _get_nc(use_mask: bool = False):
    if use_mask not in _CACHE:
        _CACHE[use_mask] = _build(use_mask)
    return _CACHE[use_mask]


def kernel(hidden_states, attention_mask, Wq, bq, Wk, bk, Wv, bv):
    xT = np.ascontiguousarray(
        np.asarray(hidden_states, dtype=np.float32).reshape(BS, D).T.astype(
            np.float16
        )
    )
    mask = np.ascontiguousarray(np.asarray(attention_mask, dtype=np.float32)).reshape(
        B, S
    )
    Wq = np.asarray(Wq, dtype=np.float16)
    Wk = np.asarray(Wk, dtype=np.float16)
    Wv = np.asarray(Wv, dtype=np.float16)
    bq = np.asarray(bq, dtype=np.float32)
    bk = np.asarray(bk, dtype=np.float32)
    bv = np.asarray(bv, dtype=np.float32)

    nc = _get_nc(bool(np.any(mask)))

    in_maps = []
    for c in range(N_CORES):
        sl = slice(c * DPC, (c + 1) * DPC)
        in_maps.append(
            {
                "xt": xT,
                "wq": np.ascontiguousarray(Wq[:, sl]),
                "wk": np.ascontiguousarray(Wk[:, sl]),
                "wv": np.ascontiguousarray(Wv[:, sl]),
                "bq": np.ascontiguousarray(bq[sl]),
                "bk": np.ascontiguousarray(bk[sl]),
                "msk": mask,
            }
        )

    res = run_bass_kernel_spmd(nc, in_maps, core_ids=list(range(N_CORES)))
    parts = [res.results[c]["out"].reshape(B, S, DPC) for c in range(N_CORES)]
    full = np.concatenate(parts, axis=2)
    if np.any(bv):
        full = full + bv  # softmax weights sum to 1: V bias is an output shift
    return full



# revision 40
# speedup vs baseline: 1.0367x; 1.0367x over previous
"""BERT self-attention on 8 Trainium2 NeuronCores (Bass/Tile), v2.

Sharding: tensor-parallel over heads. Core c owns heads {2c, 2c+1} =
output columns [128c, 128c+128). Every core reads the full (host
pre-transposed, fp16) hidden_states; the host concatenates the 8
per-core [B*S, 128] outputs and adds bv (softmax weights sum to 1, so
the V bias is a constant output shift).

Per-core pipeline (B=4, S=2048, D=1024, head_dim=64), all-fp16 matmul
operands (fp32 PSUM accumulation; measured end-to-end rel err ~1e-3):

  QKV(b):  Q^T/K^T [dpc=128, t] via stationary-W matmuls (moving X^T,
    N=512); V directly in [t, d] layout via stationary-X^T matmuls
    (moving Wv, N=128) -- no transposes anywhere. PSUM->SBUF copies on
    DVE add the q/k biases and fold exp(attention_mask) into V rows
    plus a per-head exp(mask) column for the softmax denominators.

  attn(b, h, qc):  scores S^T[k, q] as K=64 single-shot matmuls
    (N=512); exp on ACT over [128, 2, 512] PSUM tiles -> es fp16, with
    3-5 of 16 k-blocks computed instead via a phase-averaged
    Schraudolph fast-exp on DVE (one int16 magic-constant convert, an
    exact integer -512 phase shift, and an fp16 sum -- folded into PV
    as a second accumulating matmul for the last batch, whose units
    have no next-batch projection work to fill PE slack) to keep ACT
    under the PE roofline; PV with the big es operand STATIONARY
    ([128 k, 128 q]) and v_aug [128 k, 65] moving (ones column ->
    denominators); all 4 q-tile chains of a pv tile form a single
    psum accumulation group (one 2KB lazy-zero region allows only one
    open group); DVE reciprocal + broadcast-multiply into a per-batch
    staging tile; one [128, 4, 128] DMA per 512 rows (dram side
    rearranged so element orders match).

The PE instruction stream is software-pipelined at ~0.5us quantum
granularity (engines execute in order, so any instruction waiting on
a PSUM ring slot head-blocks everything behind it): each k-block slot
pops a next-batch QKV quantum and a PV stripe step (PV_LAG slots
behind exp, consuming each es tile as soon as it exists), then emits
2 score matmuls + the exp; normalize quanta are deferred behind the
PSUM-releasing converts in DVE's queue. Batch 0 runs a minimal k/q
prefix with deadline-ordered fillers; DMA issue order is matched to
the consumption chain.
"""

from collections import deque

import numpy as np

import concourse.bass as bass
import concourse.tile as tile
from concourse import bacc, mybir
from concourse.bass_utils import run_bass_kernel_spmd

B, S, D, H = 4, 2048, 1024, 16
DH = 64
N_CORES = 8
DPC = D // N_CORES  # 128 output dims (2 heads) per core
BS = B * S  # 8192

F32 = mybir.dt.float32
F16 = mybir.dt.float16
I16 = mybir.dt.int16

# Phase-averaged Schraudolph fast-exp (tuned vs numpy on the reference
# distribution): es = f16bits(round(p*A + B1)) + f16bits(round(p*A + B2))
# where p is the raw (unscaled) score. A folds the 1/sqrt(dh) scale and
# log2(e) into the fp16 exponent step; B2 = B1 - 512 phase-shifts the
# interpolation sawtooth half a period; both are shifted log2(1.7766) so
# the pair sums to an unbiased exp estimate (+-1.1% weight error).
import os

if os.environ.get("NOSCH"):
    SCHRAUD_KB = ()  # which of 16 k-blocks per (h, qc) use fast-exp
    SCHRAUD_KB_STARVED = ()
    SCHRAUD_KB_TAIL = ()
else:
    SCHRAUD_KB = (4, 9, 14)
    # the last batch's non-first units have no QKV filler left for the PE,
    # so they are exp-retire-paced: alternate exp between ACT and the
    # DVE(op1)+Pool(op2) Schraudolph path on ~half the k-blocks so the two
    # engines' retirements overlap; the phase-pair sum folds into PV as two
    # accumulating matmuls (PE slack absorbs it)
    SCHRAUD_KB_STARVED = (1, 3, 5, 7, 9, 11, 13)
    SCHRAUD_KB_TAIL = (1, 3, 5, 7, 9, 11)  # keep the final k-blocks on ACT
SCH_A = 0.125 * 1.4426950408889634 * 1024.0
SCH_B1 = 14.170916 * 1024.0
SCH_B2 = 13.670916 * 1024.0

_CACHE: dict = {}


def _build(use_mask: bool):
    nc = bacc.Bacc(
        "TRN2", target_bir_lowering=False, debug=False, enable_asserts=False
    )

    xtd = nc.dram_tensor("xt", [D, BS], F16, kind="ExternalInput").ap()
    wq = nc.dram_tensor("wq", [D, DPC], F16, kind="ExternalInput").ap()
    wk = nc.dram_tensor("wk", [D, DPC], F16, kind="ExternalInput").ap()
    wv = nc.dram_tensor("wv", [D, DPC], F16, kind="ExternalInput").ap()
    bq = nc.dram_tensor("bq", [DPC], F32, kind="ExternalInput").ap()
    bk = nc.dram_tensor("bk", [DPC], F32, kind="ExternalInput").ap()
    msk = nc.dram_tensor("msk", [B, S], F32, kind="ExternalInput").ap()
    # fp16 output halves the output bytes (values are softmax-averaged,
    # |out| < 1, so fp16 adds ~5e-4 rel); [partition, 128-token-tile, d]
    # layout keeps each descriptor 1KB-contiguous (sub-512B descriptors pay
    # a 2x DMA latency penalty). The host undoes the permutation + adds bv.
    out = nc.dram_tensor("out", [128, BS // 128, DPC], F16, kind="ExternalOutput").ap()

    Exp = mybir.ActivationFunctionType.Exp
    Add = mybir.AluOpType.add
    Mult = mybir.AluOpType.mult

    with tile.TileContext(nc) as tc:
        with (
            tc.tile_pool(name="consts", bufs=1) as consts,
            tc.tile_pool(name="p_xt", bufs=6) as p_xt,
            tc.tile_pool(name="p_qk", bufs=2) as p_qk,
            tc.tile_pool(name="p_v", bufs=2) as p_v,
            tc.tile_pool(name="p_es", bufs=33) as p_es,
            tc.tile_pool(name="p_fin", bufs=2) as p_fin,
            tc.tile_pool(name="p_sm", bufs=8) as p_sm,
            tc.tile_pool(name="ps_qk", bufs=2, space="PSUM") as ps_qk,
            tc.tile_pool(name="ps_sp", bufs=2, space="PSUM") as ps_sp,
            tc.tile_pool(name="ps_pv", bufs=2, space="PSUM") as ps_pv,
        ):
            # ---- PE warm spin: the first real matmul can't start until
            # ~3.5us of serialized DMA (wk + xt chunk); matmuls on a zeroed
            # scratch tile keep the PE busy through its p-state ramp so the
            # real work starts at full clock (results go to a psum tile that
            # the pool recycles via start=True zeroing) ----
            warm = consts.tile([128, 512], F16, tag="warm")
            nc.vector.memset(warm, 0.0)
            warm_ps = ps_sp.tile([128, 2, 512], F32, tag="sp", name="warm_ps")

            def warm_mm(n):
                for wi in range(n):
                    nc.tensor.matmul(
                        warm_ps[:, 0, :],
                        warm[:, 0:128],
                        warm,
                        start=(wi == 0),
                        stop=(wi == n - 1),
                    )

            warm_mm(6)

            # ---- DMA issue order follows the consumption order (the DMA
            # engines transfer strictly in issue order): wk + batch 0's four
            # X^T chunks first (k-projections), only then wq/wv/bias/mask ----
            wk_sb = consts.tile([128, 8, DPC], F16, tag="wk_sb")
            wk_src = wk.rearrange("(cc p) d -> p cc d", p=128)

            xts: dict = {}

            def load_xt(b, tch, half=None, quarter=False):
                """X^T chunk in two half-transfers so the first 4-matmul
                projection group can start after half a chunk (or 2-cc
                quarters for the very first chunk: the DMA-engine track is
                serial, so smaller leading transfers start the PE sooner)."""
                t0 = b * S + tch * 512
                if (b, tch) not in xts:
                    xts[(b, tch)] = p_xt.tile(
                        [128, 8, 512], F16, tag="xt", name=f"xt{b}_{tch}"
                    )
                xt = xts[(b, tch)]
                src = xtd[:, t0 : t0 + 512].rearrange("(cc p) t -> p cc t", p=128)
                if half != 1:
                    if quarter is False:
                        nc.sync.dma_start(out=xt[:, 0:4, :], in_=src[:, 0:4, :])
                    else:
                        cc = 2 * quarter
                        nc.sync.dma_start(
                            out=xt[:, cc : cc + 2, :], in_=src[:, cc : cc + 2, :]
                        )
                if half != 0:
                    nc.sync.dma_start(out=xt[:, 4:8, :], in_=src[:, 4:8, :])

            # interleave the weight/X transfers to match the k0a -> k0b ->
            # q0 -> k1 consumption chain of batch 0's projection prefix;
            # 2-cc granularity up front so the first matmuls start ~1.9us
            bq_sb = consts.tile([128, 1], F32, tag="bq_sb")
            bk_sb = consts.tile([128, 1], F32, tag="bk_sb")
            nc.sync.dma_start(out=wk_sb[:, 0:4, :], in_=wk_src[:, 0:4, :])
            load_xt(0, 0, half=0)
            nc.sync.dma_start(out=bk_sb, in_=bk.rearrange("(p o) -> p o", o=1))
            nc.sync.dma_start(out=bq_sb, in_=bq.rearrange("(p o) -> p o", o=1))
            nc.sync.dma_start(out=wk_sb[:, 4:8, :], in_=wk_src[:, 4:8, :])
            load_xt(0, 0, half=1)

            wq_sb = consts.tile([128, 8, DPC], F16, tag="wq_sb")
            wv_sb = consts.tile([128, 8, DPC], F16, tag="wv_sb")
            nc.sync.dma_start(out=wq_sb, in_=wq.rearrange("(cc p) d -> p cc d", p=128))
            load_xt(0, 1)
            nc.sync.dma_start(out=wv_sb, in_=wv.rearrange("(cc p) d -> p cc d", p=128))

            load_xt(0, 2)
            load_xt(0, 3)

            if use_mask:
                m_sb = consts.tile([128, B, 16], F32, tag="m_sb")
                nc.sync.dma_start(
                    out=m_sb, in_=msk.rearrange("b (kb p) -> p b kb", p=128)
                )
                emask = consts.tile([128, B, 16], F32, tag="emask")
                nc.scalar.activation(emask, m_sb, Exp)

            # per-b state handles
            qT: dict = {}
            kT: dict = {}
            v_sb: dict = {}
            fin: dict = {}
            accs: dict = {}

            qkv_q: deque = deque()

            def push_qkv(b):
                """Queue batch b's projection work as ~0.5us PE quanta.
                K projections first so the next attention unit's scores
                can start as early as possible."""

                def alloc(b=b):
                    qT[b] = p_qk.tile([128, 4, 512], F16, tag="qT", name=f"qT{b}")
                    kT[b] = p_qk.tile([128, 4, 512], F16, tag="kT", name=f"kT{b}")
                    v_sb[b] = p_v.tile(
                        [128, 16, 2, DH + 1], F16, tag="v_sb", name=f"v{b}"
                    )
                    # softmax-denominator / mask column for every k-tile
                    if use_mask:
                        nc.vector.tensor_copy(
                            v_sb[b][:, :, :, DH : DH + 1],
                            emask[:, b, :]
                            .unsqueeze(2)
                            .unsqueeze(3)
                            .broadcast_to([128, 16, 2, 1]),
                        )
                    else:
                        nc.vector.memset(v_sb[b][:, :, :, DH : DH + 1], 1.0)

                qkv_q.append((b, alloc))

                def proj_half(b, tch, w_sb, which, half):
                    """4 accumulating matmuls (half the D contraction)."""
                    key = (b, tch, which)
                    if half == 0:
                        accs[key] = ps_qk.tile(
                            [128, 512], F32, tag="qk", name=f"acc{which}{b}{tch}"
                        )
                    acc = accs[key]
                    xt = xts[(b, tch)]
                    for cc in range(4 * half, 4 * half + 4):
                        nc.tensor.matmul(
                            acc,
                            w_sb[:, cc, :],
                            xt[:, cc, :],
                            start=(cc == 0),
                            stop=(cc == 7),
                        )
                    if half == 1:
                        dst, b_sb = (
                            (kT[b], bk_sb) if which == "k" else (qT[b], bq_sb)
                        )
                        nc.vector.tensor_scalar_add(dst[:, tch, :], acc, b_sb)

                def v_tile(b, tch, tt):
                    """One [128 t, 128 d] V tile: 8 matmuls + masked copy.
                    All 4 t-tiles of the chunk share the tile's single 2KB
                    zero region, so they form ONE accumulation group."""
                    key = (b, tch, "v")
                    if tt == 0:
                        accs[key] = ps_qk.tile(
                            [128, 512], F32, tag="qk", name=f"accv{b}{tch}"
                        )
                    vps = accs[key]
                    xt = xts[(b, tch)]
                    o = vps[:, tt * 128 : (tt + 1) * 128]
                    for cc in range(8):
                        nc.tensor.matmul(
                            o,
                            xt[:, cc, tt * 128 : (tt + 1) * 128],
                            wv_sb[:, cc, :],
                            start=(tt == 0 and cc == 0),
                            stop=(tt == 3 and cc == 7),
                        )
                    if tt < 3:
                        return  # copies only after the psum group closes
                    t0 = tch * 4
                    if use_mask:
                        for i in range(4):
                            nc.vector.tensor_scalar_mul(
                                v_sb[b][:, t0 + i, :, 0:DH],
                                vps[:, i * 128 : (i + 1) * 128].rearrange(
                                    "p (h d) -> p h d", h=2
                                ),
                                emask[:, b, t0 + i : t0 + i + 1],
                            )
                    else:
                        for i in range(4):
                            nc.vector.tensor_copy(
                                v_sb[b][:, t0 + i, :, 0:DH],
                                vps[:, i * 128 : (i + 1) * 128].rearrange(
                                    "p (h d) -> p h d", h=2
                                ),
                            )

                if b == 0:
                    # minimal prefix for unit (0,0,0): k/q of chunks 0-1
                    # (drained upfront); everything else becomes unit-0 slot
                    # fillers ordered by first use (v0 for the PV stripe,
                    # k2/k3 before their score slots, then v1/q2/v2; q3/v3
                    # drain at unit end) so the first exp starts ~14us earlier
                    def ph(t, w, which, half):
                        return (0, lambda: proj_half(0, t, w, which, half))

                    def vt(t, tt):
                        return (0, lambda: v_tile(0, t, tt))

                    qkv_q.extend([ph(0, wk_sb, "k", 0), ph(0, wk_sb, "k", 1)])
                    qkv_q.extend([ph(0, wq_sb, "q", 0), ph(0, wq_sb, "q", 1)])
                    qkv_q.extend([ph(1, wk_sb, "k", 0), ph(1, wk_sb, "k", 1)])
                    qkv_q.extend([ph(1, wq_sb, "q", 0), ph(1, wq_sb, "q", 1)])
                    # ---- fillers from here (popped 1/slot inside unit 0,
                    # ordered by deadline: scores(kb) needs kT[kb//4] by slot
                    # kb; pv_step(kb) at slot kb+8 needs the chunk's LAST v
                    # quantum (which carries the psum->sbuf copies) first ----
                    qkv_q.extend([vt(0, tt) for tt in range(4)])  # slots 0-3
                    qkv_q.extend([ph(2, wk_sb, "k", 0), ph(2, wk_sb, "k", 1)])
                    qkv_q.extend([vt(1, tt) for tt in range(4)])  # slots 6-9
                    qkv_q.extend([ph(3, wk_sb, "k", 0), ph(3, wk_sb, "k", 1)])
                    qkv_q.extend([vt(2, tt) for tt in range(4)])  # slots 12-15
                    # drained at unit-0 end, before its post-loop pv steps:
                    qkv_q.extend([ph(2, wq_sb, "q", 0), ph(2, wq_sb, "q", 1)])
                    qkv_q.extend([ph(3, wq_sb, "q", 0), ph(3, wq_sb, "q", 1)])
                    qkv_q.extend([vt(3, tt) for tt in range(4)])
                    return
                # chunk-major so the prefetch tracks the DMA arrival order
                for tch in range(4):
                    qkv_q.append((b, lambda b=b, t=tch: load_xt(b, t)))
                    qkv_q.append((b, lambda b=b, t=tch: proj_half(b, t, wk_sb, "k", 0)))
                    qkv_q.append((b, lambda b=b, t=tch: proj_half(b, t, wk_sb, "k", 1)))
                    qkv_q.append((b, lambda b=b, t=tch: proj_half(b, t, wq_sb, "q", 0)))
                    qkv_q.append((b, lambda b=b, t=tch: proj_half(b, t, wq_sb, "q", 1)))
                    for tt in range(4):
                        qkv_q.append((b, lambda b=b, t=tch, tt=tt: v_tile(b, t, tt)))

            PV_LAG = 8  # slots between exp(kb) and its striped PV step
            dve_q: deque = deque()  # deferred normalize/ship quanta

            def emit_unit(b, h, qc, schraud, lag=PV_LAG, starved=False,
                          pop_stride=2, pop_valve=33, greedy=False,
                          ring3=False, last=False):
                """One (batch, head, 1024-q-chunk) attention unit as a
                single slot pipeline: per k-block 2 score matmuls + exp,
                the PV accumulation STRIPED kb-wise PV_LAG slots behind
                (each es tile is consumed as soon as it exists), and one
                next-batch QKV quantum per slot."""
                hp = h * DH
                if h == 0 and qc == 0:
                    fin[b] = p_fin.tile([128, 16, DPC], F16, tag="fin", name=f"f{b}")
                pvt = [
                    ps_pv.tile([128, 4, DH + 1], F32, tag="pv", name=f"pv{j}")
                    for j in range(2)
                ]
                tiles = []

                def pv_step(kb):
                    # all 4 q-tile chains of one pv tile form a SINGLE psum
                    # accumulation group (one 2KB zero region): start only on
                    # the region's very first matmul, stop on its last —
                    # later chains' first writes consume the pending-zero
                    parts = tiles[kb]
                    for qt in range(8):
                        for pi, part in enumerate(parts):
                            nc.tensor.matmul(
                                pvt[qt // 4][:, qt % 4, :],
                                part[:, qt // 4, (qt % 4) * 128 : (qt % 4 + 1) * 128],
                                v_sb[b][:, kb, h, :],
                                start=(kb == 0 and pi == 0 and qt % 4 == 0),
                                stop=(
                                    kb == 15
                                    and pi == len(parts) - 1
                                    and qt % 4 == 3
                                ),
                            )

                pv_sum = starved  # starved units: PE slack absorbs the sum
                for kb in range(16):
                    # QKV + PV quanta lead the slot so a scores matmul that
                    # must wait for its PSUM ring slot (ACT/DVE drain) sits
                    # behind ready work instead of head-blocking it (QKV pop
                    # first: unit-0 fillers write the very tiles pv_step reads)
                    if qkv_q and (kb % pop_stride == 0 or len(qkv_q) > pop_valve):
                        qkv_q.popleft()[1]()
                        if greedy and qkv_q:
                            qkv_q.popleft()[1]()
                    if kb >= lag:
                        pv_step(kb - lag)
                    # tail units have no projections running, so the idle
                    # ps_qk banks act as a third score-ring slot (as two
                    # half-kb tiles) -- the exp-retire latency chain then
                    # spans 3 slots instead of 2 and stops gating the PE
                    if ring3 and kb % 3 == 2:
                        sps = [
                            ps_qk.tile([128, 512], F32, tag="qk", name=f"sp3{kb}{j}")
                            for j in range(2)
                        ]
                    else:
                        sp = ps_sp.tile([128, 2, 512], F32, tag="sp")
                        sps = None
                    for j in range(2):
                        nc.tensor.matmul(
                            sps[j] if sps else sp[:, j, :],
                            kT[b][
                                hp : hp + DH,
                                kb // 4,
                                (kb % 4) * 128 : (kb % 4 + 1) * 128,
                            ],
                            qT[b][hp : hp + DH, 2 * qc + j, :],
                            start=True,
                            stop=True,
                        )
                    es = p_es.tile(
                        [128, 2, 512], F16, tag="es", name=f"es{kb}", bufs=14
                    )
                    if kb in schraud:
                        # one PSUM read on DVE: u1 = round(p*A + B1) as int16
                        # bits; the half-period phase shift is an exact
                        # integer subtract (B1-B2 = 512 = half the fp16
                        # exponent step), SBUF-only so it runs on the
                        # otherwise-idle Pool engine. The phase-pair sum runs
                        # on Pool too, or (starved units) folds into PV as a
                        # 2nd accumulating matmul.
                        u2 = p_es.tile(
                            [128, 2, 512], I16, tag="sch", name="u2", bufs=6
                        )
                        if sps:
                            for j in range(2):
                                nc.vector.tensor_scalar(
                                    es.bitcast(I16)[:, j, :], sps[j],
                                    SCH_A, SCH_B1, op0=Mult, op1=Add,
                                )
                        else:
                            nc.vector.tensor_scalar(
                                es.bitcast(I16), sp, SCH_A, SCH_B1,
                                op0=Mult, op1=Add,
                            )
                        nc.vector.tensor_scalar(
                            u2, es.bitcast(I16), 512, None,
                            op0=mybir.AluOpType.subtract,
                        )
                        if pv_sum:
                            tiles.append((es, u2.bitcast(F16)))
                        else:
                            nc.vector.tensor_tensor(es, es, u2.bitcast(F16), Add)
                            tiles.append((es,))
                    else:
                        if sps:
                            for j in range(2):
                                nc.scalar.activation(
                                    es[:, j, :], sps[j], Exp, scale=0.125
                                )
                        else:
                            nc.scalar.activation(es, sp, Exp, scale=0.125)
                        tiles.append((es,))
                    # deferred norms run AFTER this slot's exp ops so they
                    # never delay the PSUM-releasing converts in DVE's queue
                    if dve_q:
                        dve_q.popleft()()
                # all same-batch projection writes must be emitted before the
                # remaining PV steps read them (batch 0's deferred v tiles)
                while qkv_q and qkv_q[0][0] == b:
                    qkv_q.popleft()[1]()
                for kb in range(16 - lag, 16):
                    pv_step(kb)

                def norm_half(half, b=b, h=h, qc=qc, pvt=pvt, last=last):
                    pv = pvt[half]
                    rc = p_sm.tile([128, 4, 1], F32, tag="rc")
                    nc.vector.reciprocal(rc, pv[:, :, DH : DH + 1])
                    qi0 = 8 * qc + 4 * half
                    nc.vector.tensor_tensor(
                        fin[b][:, qi0 : qi0 + 4, h * DH : (h + 1) * DH],
                        pv[:, :, 0:DH],
                        rc.broadcast_to([128, 4, DH]),
                        Mult,
                    )
                    if h == 1:  # both heads done: ship these 4 q-tiles
                        t0 = b * 16 + qi0
                        # the very last transfer issues from ACT's idle DGE
                        # queue so the two final DMAs don't serialize their
                        # descriptor generation
                        dge = nc.scalar if (last and half == 1) else nc.sync
                        dge.dma_start(
                            out=out[:, t0 : t0 + 4, :],
                            in_=fin[b][:, qi0 : qi0 + 4, :],
                        )

                dve_q.append(lambda: norm_half(0))
                dve_q.append(lambda: norm_half(1))

            # ---- software-pipelined emission ----
            push_qkv(0)
            qkv_q.popleft()[1]()  # alloc
            qkv_q.popleft()[1]()  # k0 half a
            warm_mm(3)  # PE stays hot while xt chunk-0's tail transfers land
            qkv_q.popleft()[1]()  # k0 half b
            warm_mm(2)  # ... and while wq lands
            for _ in range(6):  # q(chunk 0) + k/q(chunk 1)
                qkv_q.popleft()[1]()
            units = [(b, h, qc) for b in range(B) for h in range(2) for qc in range(2)]
            for i, (b, h, qc) in enumerate(units):
                if h == 0 and qc == 0:
                    # batch b's projections must be fully emitted before any
                    # instruction reading qT/kT/v_sb[b] (Tile deps track
                    # writes-before-reads in program order); b=0 defers its
                    # v/q23 quanta into unit 0's slots, and the last batch
                    # keeps its late-deadline quanta as greedy fillers for
                    # its own first unit (the in-unit drain before the
                    # post-loop pv steps guarantees the write-before-read
                    # emission order for v/q chunks)
                    if 0 < b < B - 1:
                        while qkv_q and qkv_q[0][0] == b:
                            qkv_q.popleft()[1]()
                    if b + 1 < B:
                        push_qkv(b + 1)
                starved = ring3 = False
                pop_stride, pop_valve, greedy = 2, 33, False
                if b == B - 2:
                    # slow the filler drain so the last batch's first unit
                    # keeps ~20 of its projection quanta as its own fillers
                    pop_stride, pop_valve = 4, 38
                lag = PV_LAG
                if b == B - 1:
                    if h == 0 and qc == 0:
                        sch = (7, 9, 11, 13)
                        starved = True
                        pop_stride = 1
                    else:
                        # short lag in the last unit so the post-loop pv
                        # drain (and the norms behind it) starts earlier
                        sch, starved, ring3 = (2, 5, 8, 11, 14), True, True
                        if i == len(units) - 1:
                            lag = 3
                else:
                    sch = SCHRAUD_KB
                emit_unit(b, h, qc, sch, lag=8 if i == 0 else lag,
                          starved=starved, pop_stride=pop_stride,
                          pop_valve=pop_valve, greedy=greedy, ring3=ring3,
                          last=(i == len(units) - 1))
            while dve_q:
                dve_q.popleft()()
            while qkv_q:
                qkv_q.popleft()[1]()

    nc.compile()
    return nc


def _get_nc(use_mask: bool = False):
    if use_mask not in _CACHE:
        _CACHE[use_mask] = _build(use_mask)
    return _CACHE[use_mask]


def kernel(hidden_states, attention_mask, Wq, bq, Wk, bk, Wv, bv):
    xT = np.ascontiguousarray(
        np.asarray(hidden_states, dtype=np.float32).reshape(BS, D).T.astype(
            np.float16
        )
    )
    mask = np.ascontiguousarray(np.asarray(attention_mask, dtype=np.float32)).reshape(
        B, S
    )
    Wq = np.asarray(Wq, dtype=np.float16)
    Wk = np.asarray(Wk, dtype=np.float16)
    Wv = np.asarray(Wv, dtype=np.float16)
    bq = np.asarray(bq, dtype=np.float32)
    bk = np.asarray(bk, dtype=np.float32)
    bv = np.asarray(bv, dtype=np.float32)

    nc = _get_nc(bool(np.any(mask)))

    in_maps = []
    for c in range(N_CORES):
        sl = slice(c * DPC, (c + 1) * DPC)
        in_maps.append(
            {
                "xt": xT,
                "wq": np.ascontiguousarray(Wq[:, sl]),
                "wk": np.ascontiguousarray(Wk[:, sl]),
                "wv": np.ascontiguousarray(Wv[:, sl]),
                "bq": np.ascontiguousarray(bq[sl]),
                "bk": np.ascontiguousarray(bk[sl]),
                "msk": mask,
            }
        )

    res = run_bass_kernel_spmd(nc, in_maps, core_ids=list(range(N_CORES)))
    parts = [
        res.results[c]["out"]
        .astype(np.float32)
        .transpose(1, 0, 2)  # [p, tile, d] -> [tile, p, d]; token = tile*128+p
        .reshape(B, S, DPC)
        for c in range(N_CORES)
    ]
    full = np.concatenate(parts, axis=2)
    if np.any(bv):
        full = full + bv  # softmax weights sum to 1: V bias is an output shift
    return full

